# revision 1
# baseline (speedup 1.0000x reference)
"""Trainium2 Bass kernel for CrossAttention (B=2, T=S=2048, E=1024, H=16, D=64).

Sharding: 8 cores = 2 (batch) x 4 (head groups of 4 heads).
Each core computes, for its (b, g):
  - Q/K projections in feature-major layout: QT/KT = [256, 2048]
  - V projection in sequence-major layout with an appended ones column per
    head (softmax denominator comes free from the attn@V matmul)
  - causal flash-style attention:
      causally-restricted scores matmuls -> additive causal mask folded into
      PSUM via a tiny identity-lhsT matmul on the 128-wide diagonal boundary
      block -> one exp per s-block covering both heads (1024 cols per
      ACTIVATE; key-padding handled by the per-partition exp bias) ->
      causally-restricted attn@V accumulation
  - output projection partial: [1024, 2048] fp16, DMA'd out per chunk
Schedule: j-outer software pipeline; Q/K/V projections of chunk j+1 and the
o-projection of chunk j-1 are drained as background PE work between attention
steps so the PE never idles (keeps HAM at K=8/8 / 2.4 GHz).
Host: shards/transposes inputs, gathers partials, sums 4 groups per batch,
adds bo.
"""

import collections

import ml_dtypes
import numpy as np

import concourse.bass as bass
import concourse.bacc as bacc
import concourse.mybir as mybir
import concourse.tile as tile
from concourse.bass_utils import run_bass_kernel_spmd

P = 128
T = 2048          # target length
S = 2048          # source length
E = 1024          # embed dim
D = 64            # head dim
GC = 256          # channels per group (4 heads * 64)
KB = E // P       # 8 k-blocks for the E contraction
TJ = 512          # t-chunk width
NTJ = T // TJ     # 4
NSB = S // P      # 16 s-blocks
VC = 4 * (D + 1)  # 260 = V-projection cols (64 V + 1 ones per head)
SCALE = float(D) ** -0.5  # 0.125
NEG = -60000.0    # additive causal-mask constant (exp(SCALE*NEG) == 0)

F32 = mybir.dt.float32
F16 = mybir.dt.float16


def _build_program():
    nc = bacc.Bacc()

    # all inputs are partition-major (host pre-arranged) so every load below
    # is a single contiguous-per-partition DMA descriptor (~0.65us issue each;
    # multi-descriptor loads serialize on the issuing sequencer)
    xq = nc.dram_tensor("xq_t", [P, NTJ * KB * TJ], F16, kind="ExternalInput")
    xk = nc.dram_tensor("xk_t", [P, NTJ * KB * TJ], F16, kind="ExternalInput")
    xv = nc.dram_tensor("xv_t", [P, NTJ * KB * TJ], F16, kind="ExternalInput")
    wq = nc.dram_tensor("wq_t", [P, KB * GC], F16, kind="ExternalInput")
    wk = nc.dram_tensor("wk_t", [P, KB * GC], F16, kind="ExternalInput")
    wv = nc.dram_tensor("wv_t", [P, KB * VC], F16, kind="ExternalInput")
    wvb = nc.dram_tensor("wvb", [1, VC], F16, kind="ExternalInput")
    wo = nc.dram_tensor("wo_t", [P, 2 * E], F16, kind="ExternalInput")
    # itri: [:, :128] identity, [:, 128:] strictly-lower-tri * NEG
    itri = nc.dram_tensor("itri", [P, 2 * P], F16, kind="ExternalInput")
    # padb: cols 0..15 per-s-block exp bias (0 / -1e30), cols 16..19 qk biases
    padb = nc.dram_tensor("padb", [P, NSB + 4], F32, kind="ExternalInput")
    out_t = nc.dram_tensor("out_t", [E, T], F16, kind="ExternalOutput")

    with tile.TileContext(nc) as tc:
        with (
            tc.tile_pool(name="consts", bufs=1) as cpool,
            tc.tile_pool(name="xs", bufs=12) as xpool,
            tc.tile_pool(name="persist", bufs=1) as ppool,
            tc.tile_pool(name="expw", bufs=4) as epool,
            tc.tile_pool(name="norm", bufs=4) as npool,
            tc.tile_pool(name="ft", bufs=4) as fpool,
            tc.tile_pool(name="ps", bufs=1, space="PSUM") as pspool,
        ):
            # ---- constants / weights to SBUF ----
            wq_sb = cpool.tile([P, KB, GC], F16, name="wq_sb")
            wk_sb = cpool.tile([P, KB, GC], F16, name="wk_sb")
            wv_sb = cpool.tile([P, KB + 1, VC], F16, name="wv_sb")
            wo_sb = cpool.tile([P, 2, E], F16, name="wo_sb")
            itri_sb = cpool.tile([P, 2 * P], F16, name="itri_sb")
            padb_sb = cpool.tile([P, NSB + 4], F32, name="padb_sb")
            ones_sb = cpool.tile([1, P], F16, name="ones_sb")
            ones32_sb = cpool.tile([1, D], F32, name="ones32_sb")

            # Input DMAs run as three parallel need-ordered streams (one per
            # DMA-capable queue): a single queue only sustains a fraction of
            # HBM bandwidth, so the k/q/v streams ride sync/scalar/gpsimd
            # respectively. Later chunks are enqueued behind chunk 0 on the
            # same queues so they never compete with the startup-critical
            # transfers. Each load is one contiguous descriptor.
            # consts on the gpsimd queue (tiny); bulk follows on sync in
            # strict need order — serial delivery matches the PE's K->Q->V
            # consumption order, so parallel-queue dilution only hurts
            nc.gpsimd.dma_start(itri_sb[:], itri[:])
            nc.gpsimd.dma_start(padb_sb[:], padb[:])
            nc.vector.memset(ones_sb[:], 1.0)
            nc.vector.memset(ones32_sb[:], 1.0)

            # ---- persistent activations ----
            qt_sb = ppool.tile([P, 2, T], F16, name="qt_sb")
            kt_sb = ppool.tile([P, 2, S], F16, name="kt_sb")
            v_sb = ppool.tile([P, NSB, VC], F16, name="v_sb")
            aoTn = ppool.tile([P, 2, T], F16, name="aoTn")

            xts = {}

            def load_x(j, eng=None, ndesc=1):
                for nm, x_dram in (("q", xq), ("k", xk), ("v", xv)):
                    t_ = xpool.tile([P, KB, TJ], F16, tag="xs", name=f"x{nm}")
                    h = KB // ndesc
                    for hh in range(ndesc):
                        (eng or nc.sync).dma_start(
                            t_[:, h * hh : h * (hh + 1), :],
                            x_dram[
                                :,
                                j * KB * TJ
                                + hh * h * TJ : j * KB * TJ
                                + (hh + 1) * h * TJ,
                            ],
                        )
                    xts[(nm, j)] = t_

            def xslice(nm, j, kb, c0, c1):
                return xts[(nm, j)][:, kb, c0:c1]

            # warm the Exp activation table while the PE waits on DMA
            dummy = npool.tile([1, 8], F32, tag="dummy", name="dummy", bufs=1)
            nc.vector.memset(dummy[:], 0.0)
            nc.scalar.activation(
                dummy[:], dummy[:], mybir.ActivationFunctionType.Exp
            )

            # need-ordered chunk-0 k/q on the SP queue (halved so the first
            # K-proj matmuls start ~2us earlier)
            nc.sync.dma_start(wk_sb[:], wk[:])
            t_ = xpool.tile([P, KB, TJ], F16, tag="xs", name="xk")
            nc.sync.dma_start(t_[:, :4, :], xk[:, 0 : KB * TJ // 2])
            nc.sync.dma_start(t_[:, 4:, :], xk[:, KB * TJ // 2 : KB * TJ])
            xts[("k", 0)] = t_
            nc.sync.dma_start(wq_sb[:], wq[:])
            t_ = xpool.tile([P, KB, TJ], F16, tag="xs", name="xq")
            nc.sync.dma_start(t_[:], xq[:, 0 : KB * TJ])
            xts[("q", 0)] = t_
            nc.sync.dma_start(wv_sb[:, :KB, :], wv[:])
            nc.sync.dma_start(wv_sb[0:1, KB, :], wvb[:])
            t_ = xpool.tile([P, KB, TJ], F16, tag="xs", name="xv")
            nc.sync.dma_start(t_[:], xv[:, 0 : KB * TJ])
            xts[("v", 0)] = t_

            # ---------- background task machinery ----------
            bg = collections.deque()       # proj tasks: must finish before
                                           # the attention chunk that reads them
            bg_late = collections.deque()  # o-proj tasks: no deadline, kept
                                           # to fill late-chunk attention holes

            def drain(n):
                for _ in range(n):
                    if bg:
                        bg.popleft()()
                    elif bg_late:
                        bg_late.popleft()()

            def qk_proj_tasks(j):
                # channel-major Q/K projections for chunk j
                tasks = []
                # q first: attention(hp, j) gates on qt(j) for every s-block,
                # while kt(j) is only read by the last four (diagonal) blocks.
                # mc0 first: attention(hp=0, j) only reads the mc0 halves.
                for mc in range(2):
                    for nm, w_sb, dst, bcol in (
                        ("q", wq_sb, qt_sb, NSB + 2),
                        ("k", wk_sb, kt_sb, NSB),
                    ):
                        st = {}

                        def open_ps(st=st):
                            st["ps"] = pspool.tile(
                                [P, TJ], F32, tag="pr", name="ps_pr", bufs=2
                            )

                        def mm(kb0, st=st, nm=nm, w_sb=w_sb, mc=mc, j=j):
                            for kb in (kb0, kb0 + 1):
                                nc.tensor.matmul(
                                    st["ps"][:],
                                    lhsT=w_sb[:, kb, mc * P : (mc + 1) * P],
                                    rhs=xslice(nm, j, kb, 0, TJ),
                                    start=(kb == 0),
                                    stop=(kb == KB - 1),
                                )

                        def fin(st=st, dst=dst, mc=mc, j=j, bcol=bcol):
                            nc.vector.tensor_scalar_add(
                                dst[:, mc, j * TJ : (j + 1) * TJ],
                                st["ps"][:],
                                padb_sb[:, bcol + mc : bcol + mc + 1],
                            )

                        tasks.append(open_ps)
                        for kb0 in range(0, KB, 2):
                            tasks.append(lambda kb0=kb0, mm=mm: mm(kb0))
                        tasks.append(fin)
                return tasks

            def v_proj_tasks(j):
                tasks = []
                for ii in range(TJ // P):
                    i = j * (TJ // P) + ii
                    st = {}

                    def open_ps(st=st):
                        st["ps"] = pspool.tile(
                            [P, TJ], F32, tag="pr", name="ps_v", bufs=2
                        )

                    def mm(kb0, st=st, ii=ii, j=j):
                        for kb in (kb0, kb0 + 1):
                            nc.tensor.matmul(
                                st["ps"][:, :VC],
                                lhsT=xslice(nm_v, j, kb, ii * P, (ii + 1) * P),
                                rhs=wv_sb[:, kb, :],
                                start=(kb == 0),
                                stop=False,
                            )

                    def fin(st=st, i=i):
                        nc.tensor.matmul(
                            st["ps"][:, :VC],
                            lhsT=ones_sb[0:1, 0:P],
                            rhs=wv_sb[0:1, KB, :],
                            start=False,
                            stop=True,
                        )
                        nc.vector.tensor_copy(
                            out=v_sb[:, i, :], in_=st["ps"][:, :VC]
                        )

                    nm_v = "v"
                    tasks.append(open_ps)
                    for kb0 in range(0, KB, 2):
                        tasks.append(lambda kb0=kb0, mm=mm: mm(kb0))
                    tasks.append(fin)
                return tasks

            def o_proj_tasks(j, c0=0, c1=TJ):
                tasks = []
                jsl = slice(j * TJ + c0, j * TJ + c1)
                w = c1 - c0
                for mc in range(KB):
                    def task(mc=mc, jsl=jsl, w=w):
                        ps = pspool.tile([P, TJ], F32, tag="pr", name="ps_o", bufs=2)
                        for cc in range(2):
                            nc.tensor.matmul(
                                ps[:, :w],
                                lhsT=wo_sb[:, cc, mc * P : (mc + 1) * P],
                                rhs=aoTn[:, cc, jsl],
                                start=(cc == 0),
                                stop=(cc == 1),
                            )
                        oc = fpool.tile([P, TJ], F16, tag="oc", name="oc", bufs=4)
                        nc.vector.tensor_copy(out=oc[:, :w], in_=ps[:, :w])
                        # alternate queues: halves the issue-serialization of
                        # each chunk's output flush (sync is idle once the
                        # input loads finish)
                        eng = nc.gpsimd if mc % 2 else nc.sync
                        eng.dma_start(out_t[mc * P : (mc + 1) * P, jsl], oc[:, :w])
                    tasks.append(task)
                return tasks

            # ---------- attention ----------
            def attention(hp, j, c0=0, c1=TJ, ndrain=2, last=False):
                # processes t-columns [j*TJ+c0, j*TJ+c1) for head pair hp
                jsl = slice(j * TJ + c0, j * TJ + c1)
                w = c1 - c0
                # s-blocks whose causal window intersects [c0, c1)
                blocks = [
                    i for i in range(4 * j + 4) if max(0, i - 4 * j) * P < c1
                ]
                av_ps = [
                    pspool.tile([P, TJ], F32, tag=f"av{lh}", name="ps_av", bufs=1)
                    for lh in range(2)
                ]
                ets = {}

                def ci_of(i):
                    return max(c0, max(0, i - 4 * j) * P)

                def emit_scores(i):
                    ci = ci_of(i)
                    ws = max(0, i - 4 * j) * P
                    ps = pspool.tile([P, 2, TJ], F32, tag="sc", name="ps_sc", bufs=2)
                    masked = i >= 4 * j and ws >= c0
                    for lh in range(2):
                        base = D * lh
                        nc.tensor.matmul(
                            ps[:, lh, ci:c1],
                            lhsT=kt_sb[base : base + D, hp, i * P : (i + 1) * P],
                            rhs=qt_sb[base : base + D, hp, j * TJ + ci : j * TJ + c1],
                            start=True,
                            stop=not masked,
                            skip_group_check=True,
                        )
                        if masked:
                            # fold causal mask additively into PSUM:
                            # ps[:, lh, ws:ws+128] += I.T @ (NEG * lowtri)
                            nc.tensor.matmul(
                                ps[:, lh, ws : ws + P],
                                lhsT=itri_sb[:, 0:P],
                                rhs=itri_sb[:, P : 2 * P],
                                start=False,
                                stop=True,
                                skip_group_check=True,
                            )
                    return ps

                def emit_exp(i, ps):
                    ci = ci_of(i)
                    et = epool.tile([P, 2, TJ], F16, tag="exp", name="et")
                    nc.scalar.activation(
                        et[:, :, ci:c1],
                        ps[:, :, ci:c1],
                        mybir.ActivationFunctionType.Exp,
                        scale=SCALE,
                        bias=padb_sb[:, i : i + 1],
                    )
                    ets[i] = et

                def emit_av(i):
                    ci = ci_of(i)
                    et = ets.pop(i)
                    for lh in range(2):
                        h65 = (hp * 2 + lh) * (D + 1)
                        nc.tensor.matmul(
                            av_ps[lh][: D + 1, ci:c1],
                            lhsT=v_sb[:, i, h65 : h65 + D + 1],
                            rhs=et[:, lh, ci:c1],
                            start=(i == blocks[0]),
                            stop=(i == blocks[-1]),
                            skip_group_check=True,
                        )

                # software pipeline with attn@V trailing a full iteration:
                # sc(i) -> exp(i) -> sc(i+1) -> av(i-1): by the time av(i-1)
                # issues, exp(i-1) finished long ago, so the PE never waits
                # on the scalar engine in steady state
                ps_prev = emit_scores(blocks[0])
                for n, i in enumerate(blocks):
                    emit_exp(i, ps_prev)
                    drain(ndrain)
                    if n + 1 < len(blocks):
                        ps_prev = emit_scores(blocks[n + 1])
                    if n >= 1:
                        emit_av(blocks[n - 1])
                    drain(ndrain)
                emit_av(blocks[-1])

                # normalize: row D of av psum is the softmax denominator.
                # lh1 first: its extra partition-shift DMA is on the o-proj
                # critical path. DVE ops batched so FIFO heads never wait DMA.
                def emit_norm(n0, n1):
                    hw = n1 - n0
                    nsl = slice(j * TJ + n0, j * TJ + n1)
                    aoF = [
                        npool.tile([D + 1, TJ], F32, tag=f"aoF{lh}", name="aoF", bufs=2)
                        for lh in range(2)
                    ]
                    rrow = [
                        npool.tile([1, TJ], F32, tag=f"rrow{lh}", name="rrow", bufs=2)
                        for lh in range(2)
                    ]
                    rcp = [
                        npool.tile([1, TJ], F32, tag=f"rcp{lh}", name="rcp", bufs=2)
                        for lh in range(2)
                    ]
                    for lh in (1, 0):
                        nc.vector.tensor_copy(
                            out=aoF[lh][:, :hw], in_=av_ps[lh][: D + 1, n0:n1]
                        )
                    for lh in (1, 0):
                        # partition shift 64 -> 0 via SBUF DMA (no PSUM DMA)
                        nc.gpsimd.dma_start(rrow[lh][:, :hw], aoF[lh][D : D + 1, :hw])
                    for lh in (1, 0):
                        nc.vector.reciprocal_approx_fast(
                            rcp[lh][:, :hw], rrow[lh][:, :hw]
                        )
                    rb64 = [
                        npool.tile([D, TJ], F32, tag=f"rb{lh}", name="rb64", bufs=2)
                        for lh in range(2)
                    ]
                    for lh in (1, 0):
                        nc.gpsimd.partition_broadcast(
                            rb64[lh][:, :hw], rcp[lh][0:1, :hw]
                        )
                    tmp = npool.tile([D, TJ], F16, tag="aon", name="aon", bufs=2)
                    nc.vector.tensor_mul(
                        out=tmp[:, :hw], in0=aoF[1][0:D, :hw], in1=rb64[1][:, :hw]
                    )
                    # partition shift 0-63 -> 64-127 via SBUF DMA
                    nc.gpsimd.dma_start(aoTn[D : 2 * D, hp, nsl], tmp[:, :hw])
                    nc.vector.tensor_mul(
                        out=aoTn[0:D, hp, nsl],
                        in0=aoF[0][0:D, :hw],
                        in1=rb64[0][:, :hw],
                    )

                emit_norm(c0, c1)

            # ---------- main schedule ----------
            # xq(1) promoted: the q-proj(1) background tasks drain first
            # during attention(*, 0)
            t_ = xpool.tile([P, KB, TJ], F16, tag="xs", name="xq1")
            nc.sync.dma_start(t_[:], xq[:, KB * TJ : 2 * KB * TJ])
            xts[("q", 1)] = t_
            for nm, x_dram in (("k", xk), ("v", xv)):
                t_ = xpool.tile([P, KB, TJ], F16, tag="xs", name=f"x{nm}1")
                nc.sync.dma_start(t_[:], x_dram[:, KB * TJ : 2 * KB * TJ])
                xts[(nm, 1)] = t_
            nc.sync.dma_start(wo_sb[:], wo[:])
            for j in range(2, NTJ):
                load_x(j)
            for t in qk_proj_tasks(0):
                t()
            for t in v_proj_tasks(0):
                t()
            for j in range(NTJ):
                # all proj(j) tasks must be emitted before attention(j) reads
                # qt/kt/v (the PE executes its queue in order)
                drain(len(bg))
                if j + 1 < NTJ:
                    for t in qk_proj_tasks(j + 1):
                        bg.append(t)
                    for t in v_proj_tasks(j + 1):
                        bg.append(t)
                # j=0: chunk-1 x DMAs land ~19-26us, overlapping attention
                # (0,0) — drain lightly there, heavily during attention(1,0)
                attention(0, j, ndrain=(1 if j == 0 else 2))
                attention(1, j, ndrain=(4 if j == 0 else 2), last=(j == NTJ - 1))
                for t in o_proj_tasks(j):
                    bg_late.append(t)
            drain(len(bg) + len(bg_late))

    nc.compile()
    return nc


_NC_CACHE = None


def _get_nc():
    global _NC_CACHE
    if _NC_CACHE is None:
        _NC_CACHE = _build_program()
    return _NC_CACHE


def _make_in_maps(query, key, value, key_padding_mask, Wq, bq, Wk, bk, Wv, bv, Wo, bo):
    f32 = np.float32
    f16 = np.float16
    query = np.asarray(query, f32)
    key = np.asarray(key, f32)
    value = np.asarray(value, f32)
    kpm = np.asarray(key_padding_mask, bool)
    Wq, bq = np.asarray(Wq, f32), np.asarray(bq, f32)
    Wk, bk = np.asarray(Wk, f32), np.asarray(bk, f32)
    Wv, bv = np.asarray(Wv, f32), np.asarray(bv, f32)
    Wo = np.asarray(Wo, f32)

    # constants shared by all cores: identity | strictly-lower-tri * NEG
    ident = np.eye(P, dtype=f16)
    lowtri = (np.arange(P)[None, :] < np.arange(P)[:, None]).astype(f16) * f16(NEG)
    itri_np = np.concatenate([ident, lowtri], axis=1)

    def pmajor(a, inner):
        # [KB*P, inner_cols] -> partition-major [P, KB*inner_cols]
        kb = a.shape[0] // P
        return np.ascontiguousarray(
            a.reshape(kb, P, -1).transpose(1, 0, 2).reshape(P, -1)
        )

    def xprep(x):
        # [T, E] f32 -> [E, T] f16 -> chunk-major [P, NTJ*KB*TJ]
        xt = x.T.astype(f16)
        return np.ascontiguousarray(
            xt.reshape(KB, P, NTJ, TJ).transpose(1, 2, 0, 3).reshape(P, -1)
        )

    in_maps = []
    for c in range(8):
        b, g = divmod(c, 4)
        cols = slice(g * GC, (g + 1) * GC)

        wq_t = pmajor(Wq[cols, :].T.astype(f16), GC)
        wk_t = pmajor(Wk[cols, :].T.astype(f16), GC)

        wv_full = np.zeros((E, VC), f16)
        wvb_np = np.zeros((1, VC), f16)
        for h in range(4):
            ch = slice(g * GC + h * D, g * GC + (h + 1) * D)
            wv_full[:, h * (D + 1) : h * (D + 1) + D] = Wv[ch, :].T
            wvb_np[0, h * (D + 1) : h * (D + 1) + D] = bv[ch]
            wvb_np[0, h * (D + 1) + D] = 1.0  # ones col -> softmax denominator
        wv_t = pmajor(wv_full, VC)

        wo_t = pmajor(Wo[:, cols].T.astype(f16), E)

        padb_np = np.where(kpm[b], -1.0e30, 0.0).astype(f32).reshape(NSB, P).T
        biases = np.stack(
            [bk[cols][:P], bk[cols][P:], bq[cols][:P], bq[cols][P:]], axis=1
        ).astype(f32)
        padb_np = np.ascontiguousarray(np.concatenate([padb_np, biases], axis=1))

        in_maps.append(
            {
                "xq_t": xprep(query[b]),
                "xk_t": xprep(key[b]),
                "xv_t": xprep(value[b]),
                "wq_t": wq_t,
                "wk_t": wk_t,
                "wv_t": wv_t,
                "wvb": wvb_np,
                "wo_t": wo_t,
                "itri": itri_np,
                "padb": padb_np,
            }
        )
    return in_maps


def kernel(**inputs) -> np.ndarray:
    nc = _get_nc()
    in_maps = _make_in_maps(**inputs)
    res = run_bass_kernel_spmd(nc, in_maps, core_ids=list(range(8)))
    bo = np.asarray(inputs["bo"], np.float32)
    B = inputs["query"].shape[0]
    out = np.zeros((B, T, E), np.float32)
    for c in range(8):
        b = c // 4
        out[b] += res.results[c]["out_t"].T.astype(np.float32)
    out += bo[None, None, :]
    return out



# revision 11
# speedup vs baseline: 1.0753x; 1.0753x over previous
"""Trainium2 Bass kernel for CrossAttention (B=2, T=S=2048, E=1024, H=16, D=64).

Sharding: 8 cores = 2 (batch) x 4 (head groups of 4 heads).
Each core computes, for its (b, g):
  - Q/K projections in feature-major layout: QT/KT = [256, 2048]
  - V projection in sequence-major layout with an appended ones column per
    head (softmax denominator comes free from the attn@V matmul)
  - causal flash-style attention:
      causally-restricted scores matmuls -> additive causal mask folded into
      PSUM via a tiny identity-lhsT matmul on the 128-wide diagonal boundary
      block -> one exp per s-block covering both heads (1024 cols per
      ACTIVATE; key-padding handled by the per-partition exp bias) ->
      causally-restricted attn@V accumulation
  - output projection partial: [1024, 2048] fp16, DMA'd out per chunk
Schedule: j-outer software pipeline; Q/K/V projections of chunk j+1 and the
o-projection of chunk j-1 are drained as background PE work between attention
steps so the PE never idles (keeps HAM at K=8/8 / 2.4 GHz).
Host: shards/transposes inputs, gathers partials, sums 4 groups per batch,
adds bo.
"""

import collections

import ml_dtypes
import numpy as np

import concourse.bass as bass
import concourse.bacc as bacc
import concourse.mybir as mybir
import concourse.tile as tile
from concourse.bass_utils import run_bass_kernel_spmd

P = 128
T = 2048          # target length
S = 2048          # source length
E = 1024          # embed dim
D = 64            # head dim
GC = 256          # channels per group (4 heads * 64)
KB = E // P       # 8 k-blocks for the E contraction
TJ = 512          # t-chunk width
NTJ = T // TJ     # 4
NSB = S // P      # 16 s-blocks
VC = 4 * (D + 1)  # 260 = V-projection cols (64 V + 1 ones per head)
SCALE = float(D) ** -0.5  # 0.125
NEG = -60000.0    # additive causal-mask constant (exp(SCALE*NEG) == 0)

F32 = mybir.dt.float32
F16 = mybir.dt.float16


def _build_program():
    nc = bacc.Bacc()

    # all inputs are partition-major (host pre-arranged) so every load below
    # is a single contiguous-per-partition DMA descriptor (~0.65us issue each;
    # multi-descriptor loads serialize on the issuing sequencer)
    xq = nc.dram_tensor("xq_t", [P, NTJ * KB * TJ], F16, kind="ExternalInput")
    xk = nc.dram_tensor("xk_t", [P, NTJ * KB * TJ], F16, kind="ExternalInput")
    xv = nc.dram_tensor("xv_t", [P, NTJ * KB * TJ], F16, kind="ExternalInput")
    wq = nc.dram_tensor("wq_t", [P, KB * GC], F16, kind="ExternalInput")
    wk = nc.dram_tensor("wk_t", [P, KB * GC], F16, kind="ExternalInput")
    wv = nc.dram_tensor("wv_t", [P, KB * VC], F16, kind="ExternalInput")
    wvb = nc.dram_tensor("wvb", [1, VC], F32, kind="ExternalInput")
    wo = nc.dram_tensor("wo_t", [P, 2 * E], F16, kind="ExternalInput")
    # padb: cols 0..15 per-s-block exp bias (0 / -1e30), cols 16..19 qk biases
    padb = nc.dram_tensor("padb", [P, NSB + 4], F32, kind="ExternalInput")
    out_t = nc.dram_tensor("out_t", [E, T], F16, kind="ExternalOutput")

    with tile.TileContext(nc) as tc:
        with (
            tc.tile_pool(name="consts", bufs=1) as cpool,
            tc.tile_pool(name="xs", bufs=12) as xpool,
            tc.tile_pool(name="persist", bufs=1) as ppool,
            tc.tile_pool(name="expw", bufs=4) as epool,
            tc.tile_pool(name="norm", bufs=4) as npool,
            tc.tile_pool(name="ft", bufs=4) as fpool,
            tc.tile_pool(name="ps", bufs=1, space="PSUM") as pspool,
        ):
            # ---- constants / weights to SBUF ----
            wq_sb = cpool.tile([P, KB, GC], F16, name="wq_sb")
            wk_sb = cpool.tile([P, KB, GC], F16, name="wk_sb")
            wv_sb = cpool.tile([P, KB, VC], F16, name="wv_sb")
            wo_sb = cpool.tile([P, 2, E], F16, name="wo_sb")
            padb_sb = cpool.tile([P, NSB + 4], F32, name="padb_sb")
            wvb_sb = cpool.tile([1, VC], F32, name="wvb_sb")
            # bv (and the 1.0 denominator-column entries) broadcast across all
            # 128 t-partitions: added to the V-proj PSUM by the DVE instead of
            # spending PE columns on a ones-row matmul
            vbias_sb = cpool.tile([P, VC], F32, name="vbias_sb")

            # Input DMAs run as three parallel need-ordered streams (one per
            # DMA-capable queue): a single queue only sustains a fraction of
            # HBM bandwidth, so the k/q/v streams ride sync/scalar/gpsimd
            # respectively. Later chunks are enqueued behind chunk 0 on the
            # same queues so they never compete with the startup-critical
            # transfers. Each load is one contiguous descriptor.
            # consts on the gpsimd queue (tiny); bulk follows on sync in
            # strict need order — serial delivery matches the PE's K->Q->V
            # consumption order, so parallel-queue dilution only hurts
            nc.gpsimd.dma_start(padb_sb[:], padb[:])
            nc.gpsimd.dma_start(wvb_sb[:], wvb[:])
            nc.gpsimd.partition_broadcast(vbias_sb[:], wvb_sb[0:1, :])

            # ---- persistent activations ----
            qt_sb = ppool.tile([P, 2, T], F16, name="qt_sb")
            kt_sb = ppool.tile([P, 2, S], F16, name="kt_sb")
            v_sb = ppool.tile([P, NSB, VC], F16, name="v_sb")
            aoTn = ppool.tile([P, 2, T], F16, name="aoTn")

            xts = {}

            def load_x(j, eng=None, ndesc=1):
                for nm, x_dram in (("q", xq), ("k", xk), ("v", xv)):
                    t_ = xpool.tile([P, KB, TJ], F16, tag="xs", name=f"x{nm}")
                    h = KB // ndesc
                    for hh in range(ndesc):
                        (eng or nc.sync).dma_start(
                            t_[:, h * hh : h * (hh + 1), :],
                            x_dram[
                                :,
                                j * KB * TJ
                                + hh * h * TJ : j * KB * TJ
                                + (hh + 1) * h * TJ,
                            ],
                        )
                    xts[(nm, j)] = t_

            def xslice(nm, j, kb, c0, c1):
                return xts[(nm, j)][:, kb, c0:c1]

            # warm the Exp activation table while the PE waits on DMA
            dummy = npool.tile([1, 8], F32, tag="dummy", name="dummy", bufs=1)
            nc.vector.memset(dummy[:], 0.0)
            nc.scalar.activation(
                dummy[:], dummy[:], mybir.ActivationFunctionType.Exp
            )

            # need-ordered chunk-0 k/q on the SP queue (halved so the first
            # K-proj matmuls start ~2us earlier)
            nc.sync.dma_start(wk_sb[:], wk[:])
            t_ = xpool.tile([P, KB, TJ], F16, tag="xs", name="xk")
            nc.sync.dma_start(t_[:, :4, :], xk[:, 0 : KB * TJ // 2])
            nc.sync.dma_start(t_[:, 4:, :], xk[:, KB * TJ // 2 : KB * TJ])
            xts[("k", 0)] = t_
            nc.sync.dma_start(wq_sb[:], wq[:])
            t_ = xpool.tile([P, KB, TJ], F16, tag="xs", name="xq")
            nc.sync.dma_start(t_[:], xq[:, 0 : KB * TJ])
            xts[("q", 0)] = t_
            nc.sync.dma_start(wv_sb[:], wv[:])
            t_ = xpool.tile([P, KB, TJ], F16, tag="xs", name="xv")
            nc.sync.dma_start(t_[:], xv[:, 0 : KB * TJ])
            xts[("v", 0)] = t_

            # ---------- background task machinery ----------
            bg = collections.deque()       # proj tasks: must finish before
                                           # the attention chunk that reads them
            bg_late = collections.deque()  # o-proj tasks: no deadline, kept
                                           # to fill late-chunk attention holes

            def drain(n):
                for _ in range(n):
                    if bg:
                        bg.popleft()()
                    elif bg_late:
                        bg_late.popleft()()

            def qk_proj_tasks(j):
                # channel-major Q/K projections for chunk j
                tasks = []
                # q first: attention(hp, j) gates on qt(j) for every s-block,
                # while kt(j) is only read by the last four (diagonal) blocks.
                # mc0 first: attention(hp=0, j) only reads the mc0 halves.
                for mc in range(2):
                    for nm, w_sb, dst, bcol in (
                        ("q", wq_sb, qt_sb, NSB + 2),
                        ("k", wk_sb, kt_sb, NSB),
                    ):
                        st = {}

                        def open_ps(st=st):
                            st["ps"] = pspool.tile(
                                [P, TJ], F32, tag="pr", name="ps_pr", bufs=2
                            )

                        def mm(kb0, st=st, nm=nm, w_sb=w_sb, mc=mc, j=j):
                            for kb in (kb0, kb0 + 1):
                                nc.tensor.matmul(
                                    st["ps"][:],
                                    lhsT=w_sb[:, kb, mc * P : (mc + 1) * P],
                                    rhs=xslice(nm, j, kb, 0, TJ),
                                    start=(kb == 0),
                                    stop=(kb == KB - 1),
                                )

                        def fin(st=st, dst=dst, mc=mc, j=j, bcol=bcol):
                            nc.vector.tensor_scalar_add(
                                dst[:, mc, j * TJ : (j + 1) * TJ],
                                st["ps"][:],
                                padb_sb[:, bcol + mc : bcol + mc + 1],
                            )

                        tasks.append(open_ps)
                        for kb0 in range(0, KB, 2):
                            tasks.append(lambda kb0=kb0, mm=mm: mm(kb0))
                        tasks.append(fin)
                return tasks

            def v_proj_tasks(j):
                tasks = []
                for ii in range(TJ // P):
                    i = j * (TJ // P) + ii
                    st = {}

                    def open_ps(st=st):
                        st["ps"] = pspool.tile(
                            [P, TJ], F32, tag="pr", name="ps_v", bufs=2
                        )

                    def mm(kb0, st=st, ii=ii, j=j):
                        for kb in (kb0, kb0 + 1):
                            nc.tensor.matmul(
                                st["ps"][:, :VC],
                                lhsT=xslice(nm_v, j, kb, ii * P, (ii + 1) * P),
                                rhs=wv_sb[:, kb, :],
                                start=(kb == 0),
                                stop=(kb == KB - 1),
                            )

                    def fin(st=st, i=i):
                        nc.vector.tensor_add(
                            out=v_sb[:, i, :],
                            in0=st["ps"][:, :VC],
                            in1=vbias_sb[:],
                        )

                    nm_v = "v"
                    tasks.append(open_ps)
                    for kb0 in range(0, KB, 2):
                        tasks.append(lambda kb0=kb0, mm=mm: mm(kb0))
                    tasks.append(fin)
                return tasks

            def o_proj_tasks(j, c0=0, c1=TJ):
                tasks = []
                jsl = slice(j * TJ + c0, j * TJ + c1)
                w = c1 - c0
                for mc in range(KB):
                    def task(mc=mc, jsl=jsl, w=w):
                        ps = pspool.tile([P, TJ], F32, tag="pr", name="ps_o", bufs=2)
                        for cc in range(2):
                            nc.tensor.matmul(
                                ps[:, :w],
                                lhsT=wo_sb[:, cc, mc * P : (mc + 1) * P],
                                rhs=aoTn[:, cc, jsl],
                                start=(cc == 0),
                                stop=(cc == 1),
                            )
                        oc = fpool.tile([P, TJ], F16, tag="oc", name="oc", bufs=4)
                        nc.vector.tensor_copy(out=oc[:, :w], in_=ps[:, :w])
                        # alternate queues: halves the issue-serialization of
                        # each chunk's output flush (sync is idle once the
                        # input loads finish)
                        eng = nc.gpsimd if mc % 2 else nc.sync
                        eng.dma_start(out_t[mc * P : (mc + 1) * P, jsl], oc[:, :w])
                    tasks.append(task)
                return tasks

            # ---------- attention ----------
            def attention(hp, j, c0=0, c1=TJ, ndrain=2, last=False):
                # processes t-columns [j*TJ+c0, j*TJ+c1) for head pair hp
                jsl = slice(j * TJ + c0, j * TJ + c1)
                w = c1 - c0
                # s-blocks whose causal window intersects [c0, c1)
                blocks = [
                    i for i in range(4 * j + 4) if max(0, i - 4 * j) * P < c1
                ]
                av_ps = [
                    pspool.tile([P, TJ], F32, tag=f"av{lh}", name="ps_av", bufs=1)
                    for lh in range(2)
                ]
                ets = {}

                def ci_of(i):
                    return max(c0, max(0, i - 4 * j) * P)

                def emit_scores(i):
                    ci = ci_of(i)
                    ps = pspool.tile([P, 2, TJ], F32, tag="sc", name="ps_sc", bufs=2)
                    for lh in range(2):
                        base = D * lh
                        nc.tensor.matmul(
                            ps[:, lh, ci:c1],
                            lhsT=kt_sb[base : base + D, hp, i * P : (i + 1) * P],
                            rhs=qt_sb[base : base + D, hp, j * TJ + ci : j * TJ + c1],
                            start=True,
                            stop=True,
                            skip_group_check=True,
                        )
                    return ps

                def emit_exp(i, ps):
                    ci = ci_of(i)
                    ws = max(0, i - 4 * j) * P
                    et = epool.tile([P, 2, TJ], F16, tag="exp", name="et")
                    nc.scalar.activation(
                        et[:, :, ci:c1],
                        ps[:, :, ci:c1],
                        mybir.ActivationFunctionType.Exp,
                        scale=SCALE,
                        bias=padb_sb[:, i : i + 1],
                    )
                    if i >= 4 * j and ws >= c0:
                        # zero the causal upper triangle (s > t) of the
                        # diagonal 128-block post-exp, both heads in one DVE op
                        nc.gpsimd.affine_select(
                            out=et[:, :, ws : ws + P],
                            in_=et[:, :, ws : ws + P],
                            compare_op=mybir.AluOpType.is_ge,
                            fill=0.0,
                            base=0,
                            pattern=[[0, 2], [1, P]],
                            channel_multiplier=-1,
                        )
                    ets[i] = et

                def emit_av(i):
                    ci = ci_of(i)
                    et = ets.pop(i)
                    for lh in range(2):
                        h65 = (hp * 2 + lh) * (D + 1)
                        nc.tensor.matmul(
                            av_ps[lh][: D + 1, ci:c1],
                            lhsT=v_sb[:, i, h65 : h65 + D + 1],
                            rhs=et[:, lh, ci:c1],
                            start=(i == blocks[0]),
                            stop=(i == blocks[-1]),
                            skip_group_check=True,
                        )

                # software pipeline with attn@V trailing a full iteration:
                # sc(i) -> exp(i) -> sc(i+1) -> av(i-1): by the time av(i-1)
                # issues, exp(i-1) finished long ago, so the PE never waits
                # on the scalar engine in steady state
                ps_prev = emit_scores(blocks[0])
                for n, i in enumerate(blocks):
                    emit_exp(i, ps_prev)
                    drain(ndrain)
                    if n + 1 < len(blocks):
                        ps_prev = emit_scores(blocks[n + 1])
                    if n >= 1:
                        emit_av(blocks[n - 1])
                    drain(ndrain)
                emit_av(blocks[-1])

                # normalize: row D of av psum is the softmax denominator.
                # lh1 first: its extra partition-shift DMA is on the o-proj
                # critical path. DVE ops batched so FIFO heads never wait DMA.
                def emit_norm(n0, n1):
                    hw = n1 - n0
                    nsl = slice(j * TJ + n0, j * TJ + n1)
                    aoF = [
                        npool.tile([D + 1, TJ], F32, tag=f"aoF{lh}", name="aoF", bufs=2)
                        for lh in range(2)
                    ]
                    rrow = [
                        npool.tile([1, TJ], F32, tag=f"rrow{lh}", name="rrow", bufs=2)
                        for lh in range(2)
                    ]
                    rcp = [
                        npool.tile([1, TJ], F32, tag=f"rcp{lh}", name="rcp", bufs=2)
                        for lh in range(2)
                    ]
                    for lh in (1, 0):
                        nc.vector.tensor_copy(
                            out=aoF[lh][:, :hw], in_=av_ps[lh][: D + 1, n0:n1]
                        )
                    for lh in (1, 0):
                        # partition shift 64 -> 0 via SBUF DMA (no PSUM DMA)
                        nc.gpsimd.dma_start(rrow[lh][:, :hw], aoF[lh][D : D + 1, :hw])
                    for lh in (1, 0):
                        nc.vector.reciprocal_approx_fast(
                            rcp[lh][:, :hw], rrow[lh][:, :hw]
                        )
                    rb64 = [
                        npool.tile([D, TJ], F32, tag=f"rb{lh}", name="rb64", bufs=2)
                        for lh in range(2)
                    ]
                    for lh in (1, 0):
                        nc.gpsimd.partition_broadcast(
                            rb64[lh][:, :hw], rcp[lh][0:1, :hw]
                        )
                    tmp = npool.tile([D, TJ], F16, tag="aon", name="aon", bufs=2)
                    nc.vector.tensor_mul(
                        out=tmp[:, :hw], in0=aoF[1][0:D, :hw], in1=rb64[1][:, :hw]
                    )
                    # partition shift 0-63 -> 64-127 via SBUF DMA
                    nc.gpsimd.dma_start(aoTn[D : 2 * D, hp, nsl], tmp[:, :hw])
                    nc.vector.tensor_mul(
                        out=aoTn[0:D, hp, nsl],
                        in0=aoF[0][0:D, :hw],
                        in1=rb64[0][:, :hw],
                    )

                emit_norm(c0, c1)

            # ---------- main schedule ----------
            # xq(1) promoted: the q-proj(1) background tasks drain first
            # during attention(*, 0)
            t_ = xpool.tile([P, KB, TJ], F16, tag="xs", name="xq1")
            nc.sync.dma_start(t_[:], xq[:, KB * TJ : 2 * KB * TJ])
            xts[("q", 1)] = t_
            for nm, x_dram in (("k", xk), ("v", xv)):
                t_ = xpool.tile([P, KB, TJ], F16, tag="xs", name=f"x{nm}1")
                nc.sync.dma_start(t_[:], x_dram[:, KB * TJ : 2 * KB * TJ])
                xts[(nm, 1)] = t_
            nc.sync.dma_start(wo_sb[:], wo[:])
            for j in range(2, NTJ):
                load_x(j)
            for t in qk_proj_tasks(0):
                t()
            for t in v_proj_tasks(0):
                t()
            for j in range(NTJ):
                # all proj(j) tasks must be emitted before attention(j) reads
                # qt/kt/v (the PE executes its queue in order)
                drain(len(bg))
                if j + 1 < NTJ:
                    for t in qk_proj_tasks(j + 1):
                        bg.append(t)
                    for t in v_proj_tasks(j + 1):
                        bg.append(t)
                # j=0: chunk-1 x DMAs land ~19-26us, overlapping attention
                # (0,0) — drain lightly there, heavily during attention(1,0)
                attention(0, j, ndrain=(1 if j == 0 else 2))
                attention(1, j, ndrain=(4 if j == 0 else 2), last=(j == NTJ - 1))
                for t in o_proj_tasks(j):
                    bg_late.append(t)
            drain(len(bg) + len(bg_late))

    nc.compile()
    return nc


_NC_CACHE = None


def _get_nc():
    global _NC_CACHE
    if _NC_CACHE is None:
        _NC_CACHE = _build_program()
    return _NC_CACHE


def _make_in_maps(query, key, value, key_padding_mask, Wq, bq, Wk, bk, Wv, bv, Wo, bo):
    f32 = np.float32
    f16 = np.float16
    query = np.asarray(query, f32)
    key = np.asarray(key, f32)
    value = np.asarray(value, f32)
    kpm = np.asarray(key_padding_mask, bool)
    Wq, bq = np.asarray(Wq, f32), np.asarray(bq, f32)
    Wk, bk = np.asarray(Wk, f32), np.asarray(bk, f32)
    Wv, bv = np.asarray(Wv, f32), np.asarray(bv, f32)
    Wo = np.asarray(Wo, f32)

    def pmajor(a, inner):
        # [KB*P, inner_cols] -> partition-major [P, KB*inner_cols]
        kb = a.shape[0] // P
        return np.ascontiguousarray(
            a.reshape(kb, P, -1).transpose(1, 0, 2).reshape(P, -1)
        )

    def xprep(x):
        # [T, E] f32 -> [E, T] f16 -> chunk-major [P, NTJ*KB*TJ]
        xt = x.T.astype(f16)
        return np.ascontiguousarray(
            xt.reshape(KB, P, NTJ, TJ).transpose(1, 2, 0, 3).reshape(P, -1)
        )

    in_maps = []
    for c in range(8):
        b, g = divmod(c, 4)
        cols = slice(g * GC, (g + 1) * GC)

        wq_t = pmajor(Wq[cols, :].T.astype(f16), GC)
        wk_t = pmajor(Wk[cols, :].T.astype(f16), GC)

        wv_full = np.zeros((E, VC), f16)
        wvb_np = np.zeros((1, VC), f32)
        for h in range(4):
            ch = slice(g * GC + h * D, g * GC + (h + 1) * D)
            wv_full[:, h * (D + 1) : h * (D + 1) + D] = Wv[ch, :].T
            wvb_np[0, h * (D + 1) : h * (D + 1) + D] = bv[ch]
            wvb_np[0, h * (D + 1) + D] = 1.0  # ones col -> softmax denominator
        wv_t = pmajor(wv_full, VC)

        wo_t = pmajor(Wo[:, cols].T.astype(f16), E)

        padb_np = np.where(kpm[b], -1.0e30, 0.0).astype(f32).reshape(NSB, P).T
        biases = np.stack(
            [bk[cols][:P], bk[cols][P:], bq[cols][:P], bq[cols][P:]], axis=1
        ).astype(f32)
        padb_np = np.ascontiguousarray(np.concatenate([padb_np, biases], axis=1))

        in_maps.append(
            {
                "xq_t": xprep(query[b]),
                "xk_t": xprep(key[b]),
                "xv_t": xprep(value[b]),
                "wq_t": wq_t,
                "wk_t": wk_t,
                "wv_t": wv_t,
                "wvb": wvb_np,
                "wo_t": wo_t,
                "padb": padb_np,
            }
        )
    return in_maps


def kernel(**inputs) -> np.ndarray:
    nc = _get_nc()
    in_maps = _make_in_maps(**inputs)
    res = run_bass_kernel_spmd(nc, in_maps, core_ids=list(range(8)))
    bo = np.asarray(inputs["bo"], np.float32)
    B = inputs["query"].shape[0]
    out = np.zeros((B, T, E), np.float32)
    for c in range(8):
        b = c // 4
        out[b] += res.results[c]["out_t"].T.astype(np.float32)
    out += bo[None, None, :]
    return out



# revision 18
# speedup vs baseline: 1.0961x; 1.0193x over previous
"""Trainium2 Bass kernel for CrossAttention (B=2, T=S=2048, E=1024, H=16, D=64).

Sharding: 8 cores = 2 (batch) x 4 (head groups of 4 heads).
Each core computes, for its (b, g):
  - Q/K projections in feature-major layout: QT/KT = [256, 2048]
  - V projection in sequence-major layout with an appended ones column per
    head (softmax denominator comes free from the attn@V matmul)
  - causal flash-style attention:
      causally-restricted scores matmuls -> additive causal mask folded into
      PSUM via a tiny identity-lhsT matmul on the 128-wide diagonal boundary
      block -> one exp per s-block covering both heads (1024 cols per
      ACTIVATE; key-padding handled by the per-partition exp bias) ->
      causally-restricted attn@V accumulation
  - output projection partial: [1024, 2048] fp16, DMA'd out per chunk
Schedule: j-outer software pipeline; Q/K/V projections of chunk j+1 and the
o-projection of chunk j-1 are drained as background PE work between attention
steps so the PE never idles (keeps HAM at K=8/8 / 2.4 GHz).
Host: shards/transposes inputs, gathers partials, sums 4 groups per batch,
adds bo.
"""

import collections

import ml_dtypes
import numpy as np

import concourse.bass as bass
import concourse.bacc as bacc
import concourse.mybir as mybir
import concourse.tile as tile
from concourse.bass_utils import run_bass_kernel_spmd

P = 128
T = 2048          # target length
S = 2048          # source length
E = 1024          # embed dim
D = 64            # head dim
GC = 256          # channels per group (4 heads * 64)
KB = E // P       # 8 k-blocks for the E contraction
TJ = 512          # t-chunk width
NTJ = T // TJ     # 4
NSB = S // P      # 16 s-blocks
VC = 4 * (D + 1)  # 260 = V-projection cols (64 V + 1 ones per head)
SCALE = float(D) ** -0.5  # 0.125
NEG = -60000.0    # additive causal-mask constant (exp(SCALE*NEG) == 0)

F32 = mybir.dt.float32
F16 = mybir.dt.float16


def _build_program():
    nc = bacc.Bacc()

    # all inputs are partition-major (host pre-arranged) so every load below
    # is a single contiguous-per-partition DMA descriptor (~0.65us issue each;
    # multi-descriptor loads serialize on the issuing sequencer)
    xq = nc.dram_tensor("xq_t", [P, NTJ * KB * TJ], F16, kind="ExternalInput")
    xk = nc.dram_tensor("xk_t", [P, NTJ * KB * TJ], F16, kind="ExternalInput")
    xv = nc.dram_tensor("xv_t", [P, NTJ * KB * TJ], F16, kind="ExternalInput")
    # wq/wk are mc-major ([P, mc, kb, 128]) so each 128-out-channel half is
    # one contiguous DMA, delivered in the order the PE consumes it
    wq = nc.dram_tensor("wq_t", [P, 2 * KB * P], F16, kind="ExternalInput")
    wk = nc.dram_tensor("wk_t", [P, 2 * KB * P], F16, kind="ExternalInput")
    wv = nc.dram_tensor("wv_t", [P, KB * VC], F16, kind="ExternalInput")
    wvb = nc.dram_tensor("wvb", [1, VC], F32, kind="ExternalInput")
    wo = nc.dram_tensor("wo_t", [P, 2 * E], F16, kind="ExternalInput")
    # padb: cols 0..15 per-s-block exp bias (0 / -1e30), cols 16..19 qk biases
    padb = nc.dram_tensor("padb", [P, NSB + 4], F32, kind="ExternalInput")
    out_t = nc.dram_tensor("out_t", [E, T], F16, kind="ExternalOutput")

    with tile.TileContext(nc) as tc:
        with (
            tc.tile_pool(name="consts", bufs=1) as cpool,
            tc.tile_pool(name="xs", bufs=12) as xpool,
            tc.tile_pool(name="persist", bufs=1) as ppool,
            tc.tile_pool(name="expw", bufs=4) as epool,
            tc.tile_pool(name="norm", bufs=4) as npool,
            tc.tile_pool(name="ft", bufs=4) as fpool,
            tc.tile_pool(name="ps", bufs=1, space="PSUM") as pspool,
        ):
            # ---- constants / weights to SBUF ----
            wq_sb = cpool.tile([P, 2, KB * P], F16, name="wq_sb")
            wk_sb = cpool.tile([P, 2, KB * P], F16, name="wk_sb")
            wv_sb = cpool.tile([P, KB, VC], F16, name="wv_sb")
            wo_sb = cpool.tile([P, 2, E], F16, name="wo_sb")
            padb_sb = cpool.tile([P, NSB + 4], F32, name="padb_sb")
            wvb_sb = cpool.tile([1, VC], F32, name="wvb_sb")
            # bv (and the 1.0 denominator-column entries) broadcast across all
            # 128 t-partitions: added to the V-proj PSUM by the DVE instead of
            # spending PE columns on a ones-row matmul
            vbias_sb = cpool.tile([P, VC], F32, name="vbias_sb")

            # Input DMAs run as three parallel need-ordered streams (one per
            # DMA-capable queue): a single queue only sustains a fraction of
            # HBM bandwidth, so the k/q/v streams ride sync/scalar/gpsimd
            # respectively. Later chunks are enqueued behind chunk 0 on the
            # same queues so they never compete with the startup-critical
            # transfers. Each load is one contiguous descriptor.
            # consts on the gpsimd queue (tiny); bulk follows on sync in
            # strict need order — serial delivery matches the PE's K->Q->V
            # consumption order, so parallel-queue dilution only hurts
            nc.gpsimd.dma_start(padb_sb[:], padb[:])
            nc.gpsimd.dma_start(wvb_sb[:], wvb[:])
            nc.gpsimd.partition_broadcast(vbias_sb[:], wvb_sb[0:1, :])

            # ---- persistent activations ----
            qt_sb = ppool.tile([P, 2, T], F16, name="qt_sb")
            kt_sb = ppool.tile([P, 2, S], F16, name="kt_sb")
            v_sb = ppool.tile([P, NSB, VC], F16, name="v_sb")
            aoTn = ppool.tile([P, 2, T], F16, name="aoTn")

            xts = {}

            def load_x(j, eng=None, ndesc=1):
                for nm, x_dram in (("q", xq), ("k", xk), ("v", xv)):
                    t_ = xpool.tile([P, KB, TJ], F16, tag="xs", name=f"x{nm}")
                    h = KB // ndesc
                    for hh in range(ndesc):
                        (eng or nc.sync).dma_start(
                            t_[:, h * hh : h * (hh + 1), :],
                            x_dram[
                                :,
                                j * KB * TJ
                                + hh * h * TJ : j * KB * TJ
                                + (hh + 1) * h * TJ,
                            ],
                        )
                    xts[(nm, j)] = t_

            def xslice(nm, j, kb, c0, c1):
                return xts[(nm, j)][:, kb, c0:c1]

            # warm the Exp activation table while the PE waits on DMA
            dummy = npool.tile([1, 8], F32, tag="dummy", name="dummy", bufs=1)
            nc.vector.memset(dummy[:], 0.0)
            nc.scalar.activation(
                dummy[:], dummy[:], mybir.ActivationFunctionType.Exp
            )

            # need-ordered chunk-0 stream on the SP queue, matching the PE's
            # consumption order exactly: q-proj(mc0) -> k-proj(mc0) -> v-proj
            # run in the foreground while mc1 weights trail in; halved x loads
            # let the first matmuls start before the full tensor lands
            nc.sync.dma_start(wq_sb[:, 0, :], wq[:, 0 : KB * P])
            t_ = xpool.tile([P, KB, TJ], F16, tag="xs", name="xq")
            nc.sync.dma_start(t_[:, :4, :], xq[:, 0 : KB * TJ // 2])
            nc.sync.dma_start(t_[:, 4:, :], xq[:, KB * TJ // 2 : KB * TJ])
            xts[("q", 0)] = t_
            nc.sync.dma_start(wk_sb[:, 0, :], wk[:, 0 : KB * P])
            t_ = xpool.tile([P, KB, TJ], F16, tag="xs", name="xk")
            nc.sync.dma_start(t_[:, :4, :], xk[:, 0 : KB * TJ // 2])
            nc.sync.dma_start(t_[:, 4:, :], xk[:, KB * TJ // 2 : KB * TJ])
            xts[("k", 0)] = t_
            nc.sync.dma_start(wv_sb[:], wv[:])
            t_ = xpool.tile([P, KB, TJ], F16, tag="xs", name="xv")
            nc.sync.dma_start(t_[:], xv[:, 0 : KB * TJ])
            xts[("v", 0)] = t_
            nc.sync.dma_start(wq_sb[:, 1, :], wq[:, KB * P : 2 * KB * P])
            nc.sync.dma_start(wk_sb[:, 1, :], wk[:, KB * P : 2 * KB * P])

            # ---------- background task machinery ----------
            bg = collections.deque()       # proj tasks: must finish before
                                           # the attention chunk that reads them
            bg_late = collections.deque()  # o-proj tasks: no deadline, kept
                                           # to fill late-chunk attention holes

            def drain(n):
                for _ in range(n):
                    if bg:
                        bg.popleft()()
                    elif bg_late:
                        bg_late.popleft()()

            def qk_proj_tasks(j, mcs=(0, 1)):
                # channel-major Q/K projections for chunk j
                tasks = []
                # q first: attention(hp, j) gates on qt(j) for every s-block,
                # while kt(j) is only read by the last four (diagonal) blocks.
                # mc0 first: attention(hp=0, j) only reads the mc0 halves.
                for mc in mcs:
                    for nm, w_sb, dst, bcol in (
                        ("q", wq_sb, qt_sb, NSB + 2),
                        ("k", wk_sb, kt_sb, NSB),
                    ):
                        st = {}

                        def open_ps(st=st):
                            st["ps"] = pspool.tile(
                                [P, TJ], F32, tag="pr", name="ps_pr", bufs=2
                            )

                        def mm(kb0, st=st, nm=nm, w_sb=w_sb, mc=mc, j=j):
                            for kb in (kb0, kb0 + 1):
                                nc.tensor.matmul(
                                    st["ps"][:],
                                    lhsT=w_sb[:, mc, kb * P : (kb + 1) * P],
                                    rhs=xslice(nm, j, kb, 0, TJ),
                                    start=(kb == 0),
                                    stop=(kb == KB - 1),
                                )

                        def fin(st=st, dst=dst, mc=mc, j=j, bcol=bcol):
                            nc.vector.tensor_scalar_add(
                                dst[:, mc, j * TJ : (j + 1) * TJ],
                                st["ps"][:],
                                padb_sb[:, bcol + mc : bcol + mc + 1],
                            )

                        tasks.append(open_ps)
                        for kb0 in range(0, KB, 2):
                            tasks.append(lambda kb0=kb0, mm=mm: mm(kb0))
                        tasks.append(fin)
                return tasks

            def v_proj_tasks(j):
                tasks = []
                for ii in range(TJ // P):
                    i = j * (TJ // P) + ii
                    st = {}

                    def open_ps(st=st):
                        st["ps"] = pspool.tile(
                            [P, TJ], F32, tag="pr", name="ps_v", bufs=2
                        )

                    def mm(kb0, st=st, ii=ii, j=j):
                        for kb in (kb0, kb0 + 1):
                            nc.tensor.matmul(
                                st["ps"][:, :VC],
                                lhsT=xslice(nm_v, j, kb, ii * P, (ii + 1) * P),
                                rhs=wv_sb[:, kb, :],
                                start=(kb == 0),
                                stop=(kb == KB - 1),
                            )

                    def fin(st=st, i=i):
                        nc.vector.tensor_add(
                            out=v_sb[:, i, :],
                            in0=st["ps"][:, :VC],
                            in1=vbias_sb[:],
                        )

                    nm_v = "v"
                    tasks.append(open_ps)
                    for kb0 in range(0, KB, 2):
                        tasks.append(lambda kb0=kb0, mm=mm: mm(kb0))
                    tasks.append(fin)
                return tasks

            def o_proj_tasks(j, c0=0, c1=TJ):
                tasks = []
                jsl = slice(j * TJ + c0, j * TJ + c1)
                w = c1 - c0
                for mc in range(KB):
                    def task(mc=mc, jsl=jsl, w=w):
                        ps = pspool.tile([P, TJ], F32, tag="pr", name="ps_o", bufs=2)
                        for cc in range(2):
                            nc.tensor.matmul(
                                ps[:, :w],
                                lhsT=wo_sb[:, cc, mc * P : (mc + 1) * P],
                                rhs=aoTn[:, cc, jsl],
                                start=(cc == 0),
                                stop=(cc == 1),
                            )
                        oc = fpool.tile([P, TJ], F16, tag="oc", name="oc", bufs=4)
                        nc.vector.tensor_copy(out=oc[:, :w], in_=ps[:, :w])
                        # alternate queues: halves the issue-serialization of
                        # each chunk's output flush (sync is idle once the
                        # input loads finish)
                        eng = nc.gpsimd if mc % 2 else nc.sync
                        eng.dma_start(out_t[mc * P : (mc + 1) * P, jsl], oc[:, :w])
                    tasks.append(task)
                return tasks

            # ---------- attention ----------
            def attention(hp, j, c0=0, c1=TJ, ndrain=2, last=False):
                # processes t-columns [j*TJ+c0, j*TJ+c1) for head pair hp
                jsl = slice(j * TJ + c0, j * TJ + c1)
                w = c1 - c0
                # s-blocks whose causal window intersects [c0, c1)
                blocks = [
                    i for i in range(4 * j + 4) if max(0, i - 4 * j) * P < c1
                ]
                av_ps = [
                    pspool.tile([P, TJ], F32, tag=f"av{lh}", name="ps_av", bufs=1)
                    for lh in range(2)
                ]
                ets = {}

                def ci_of(i):
                    return max(c0, max(0, i - 4 * j) * P)

                def emit_scores(i):
                    ci = ci_of(i)
                    ps = pspool.tile([P, 2, TJ], F32, tag="sc", name="ps_sc", bufs=2)
                    for lh in range(2):
                        base = D * lh
                        nc.tensor.matmul(
                            ps[:, lh, ci:c1],
                            lhsT=kt_sb[base : base + D, hp, i * P : (i + 1) * P],
                            rhs=qt_sb[base : base + D, hp, j * TJ + ci : j * TJ + c1],
                            start=True,
                            stop=True,
                            skip_group_check=True,
                        )
                    return ps

                def emit_exp(i, ps):
                    ci = ci_of(i)
                    ws = max(0, i - 4 * j) * P
                    et = epool.tile([P, 2, TJ], F16, tag="exp", name="et")
                    nc.scalar.activation(
                        et[:, :, ci:c1],
                        ps[:, :, ci:c1],
                        mybir.ActivationFunctionType.Exp,
                        scale=SCALE,
                        bias=padb_sb[:, i : i + 1],
                    )
                    if i >= 4 * j and ws >= c0:
                        # zero the causal upper triangle (s > t) of the
                        # diagonal 128-block post-exp, both heads in one DVE op
                        nc.gpsimd.affine_select(
                            out=et[:, :, ws : ws + P],
                            in_=et[:, :, ws : ws + P],
                            compare_op=mybir.AluOpType.is_ge,
                            fill=0.0,
                            base=0,
                            pattern=[[0, 2], [1, P]],
                            channel_multiplier=-1,
                        )
                    ets[i] = et

                def emit_av(i):
                    ci = ci_of(i)
                    et = ets.pop(i)
                    for lh in range(2):
                        h65 = (hp * 2 + lh) * (D + 1)
                        nc.tensor.matmul(
                            av_ps[lh][: D + 1, ci:c1],
                            lhsT=v_sb[:, i, h65 : h65 + D + 1],
                            rhs=et[:, lh, ci:c1],
                            start=(i == blocks[0]),
                            stop=(i == blocks[-1]),
                            skip_group_check=True,
                        )

                # software pipeline with attn@V trailing a full iteration:
                # sc(i) -> exp(i) -> sc(i+1) -> av(i-1): by the time av(i-1)
                # issues, exp(i-1) finished long ago, so the PE never waits
                # on the scalar engine in steady state
                ps_prev = emit_scores(blocks[0])
                for n, i in enumerate(blocks):
                    emit_exp(i, ps_prev)
                    drain(ndrain)
                    if n + 1 < len(blocks):
                        ps_prev = emit_scores(blocks[n + 1])
                    if n >= 1:
                        emit_av(blocks[n - 1])
                    drain(ndrain)
                emit_av(blocks[-1])

                # normalize: row D of av psum is the softmax denominator.
                # lh1 first: its extra partition-shift DMA is on the o-proj
                # critical path. DVE ops batched so FIFO heads never wait DMA.
                def emit_norm(n0, n1):
                    hw = n1 - n0
                    nsl = slice(j * TJ + n0, j * TJ + n1)
                    aoF = [
                        npool.tile([D + 1, TJ], F32, tag=f"aoF{lh}", name="aoF", bufs=2)
                        for lh in range(2)
                    ]
                    rrow = [
                        npool.tile([1, TJ], F32, tag=f"rrow{lh}", name="rrow", bufs=2)
                        for lh in range(2)
                    ]
                    rcp = [
                        npool.tile([1, TJ], F32, tag=f"rcp{lh}", name="rcp", bufs=2)
                        for lh in range(2)
                    ]
                    for lh in (1, 0):
                        nc.vector.tensor_copy(
                            out=aoF[lh][:, :hw], in_=av_ps[lh][: D + 1, n0:n1]
                        )
                    for lh in (1, 0):
                        # partition shift 64 -> 0 via SBUF DMA (no PSUM DMA)
                        nc.gpsimd.dma_start(rrow[lh][:, :hw], aoF[lh][D : D + 1, :hw])
                    for lh in (1, 0):
                        nc.vector.reciprocal_approx_fast(
                            rcp[lh][:, :hw], rrow[lh][:, :hw]
                        )
                    rb64 = [
                        npool.tile([D, TJ], F32, tag=f"rb{lh}", name="rb64", bufs=2)
                        for lh in range(2)
                    ]
                    for lh in (1, 0):
                        nc.gpsimd.partition_broadcast(
                            rb64[lh][:, :hw], rcp[lh][0:1, :hw]
                        )
                    tmp = npool.tile([D, TJ], F16, tag="aon", name="aon", bufs=2)
                    nc.vector.tensor_mul(
                        out=tmp[:, :hw], in0=aoF[1][0:D, :hw], in1=rb64[1][:, :hw]
                    )
                    # partition shift 0-63 -> 64-127 via SBUF DMA
                    nc.gpsimd.dma_start(aoTn[D : 2 * D, hp, nsl], tmp[:, :hw])
                    nc.vector.tensor_mul(
                        out=aoTn[0:D, hp, nsl],
                        in0=aoF[0][0:D, :hw],
                        in1=rb64[0][:, :hw],
                    )

                emit_norm(c0, c1)

            # ---------- main schedule ----------
            # xq(1) promoted: the q-proj(1) background tasks drain first
            # during attention(*, 0)
            t_ = xpool.tile([P, KB, TJ], F16, tag="xs", name="xq1")
            nc.sync.dma_start(t_[:], xq[:, KB * TJ : 2 * KB * TJ])
            xts[("q", 1)] = t_
            for nm, x_dram in (("k", xk), ("v", xv)):
                t_ = xpool.tile([P, KB, TJ], F16, tag="xs", name=f"x{nm}1")
                nc.sync.dma_start(t_[:], x_dram[:, KB * TJ : 2 * KB * TJ])
                xts[(nm, 1)] = t_
            nc.sync.dma_start(wo_sb[:], wo[:])
            for j in range(2, NTJ):
                load_x(j)
            # foreground: only what attention(0, 0) needs (q/k mc0 + V);
            # the mc1 halves drain as background during attention(0, 0)
            for t in qk_proj_tasks(0, mcs=(0,)):
                t()
            for t in v_proj_tasks(0):
                t()
            for t in qk_proj_tasks(0, mcs=(1,)):
                bg.append(t)
            for j in range(NTJ):
                # all proj(j) tasks must be emitted before attention(j) reads
                # qt/kt/v (the PE executes its queue in order)
                if j > 0:
                    drain(len(bg))
                if j + 1 < NTJ:
                    for t in qk_proj_tasks(j + 1):
                        bg.append(t)
                    for t in v_proj_tasks(j + 1):
                        bg.append(t)
                attention(0, j, ndrain=2)
                attention(1, j, ndrain=(4 if j == 0 else 2), last=(j == NTJ - 1))
                for t in o_proj_tasks(j):
                    bg_late.append(t)
            drain(len(bg) + len(bg_late))

    nc.compile()
    return nc


_NC_CACHE = None


def _get_nc():
    global _NC_CACHE
    if _NC_CACHE is None:
        _NC_CACHE = _build_program()
    return _NC_CACHE


def _make_in_maps(query, key, value, key_padding_mask, Wq, bq, Wk, bk, Wv, bv, Wo, bo):
    f32 = np.float32
    f16 = np.float16
    query = np.asarray(query, f32)
    key = np.asarray(key, f32)
    value = np.asarray(value, f32)
    kpm = np.asarray(key_padding_mask, bool)
    Wq, bq = np.asarray(Wq, f32), np.asarray(bq, f32)
    Wk, bk = np.asarray(Wk, f32), np.asarray(bk, f32)
    Wv, bv = np.asarray(Wv, f32), np.asarray(bv, f32)
    Wo = np.asarray(Wo, f32)

    def pmajor(a, inner):
        # [KB*P, inner_cols] -> partition-major [P, KB*inner_cols]
        kb = a.shape[0] // P
        return np.ascontiguousarray(
            a.reshape(kb, P, -1).transpose(1, 0, 2).reshape(P, -1)
        )

    def xprep(x):
        # [T, E] f32 -> [E, T] f16 -> chunk-major [P, NTJ*KB*TJ]
        xt = x.T.astype(f16)
        return np.ascontiguousarray(
            xt.reshape(KB, P, NTJ, TJ).transpose(1, 2, 0, 3).reshape(P, -1)
        )

    in_maps = []
    for c in range(8):
        b, g = divmod(c, 4)
        cols = slice(g * GC, (g + 1) * GC)

        def mcmajor(a):
            # [E, GC] -> [P, mc, kb, 128] flattened to [P, 2*KB*128]
            return np.ascontiguousarray(
                a.reshape(KB, P, 2, P).transpose(1, 2, 0, 3).reshape(P, -1)
            )

        wq_t = mcmajor(Wq[cols, :].T.astype(f16))
        wk_t = mcmajor(Wk[cols, :].T.astype(f16))

        wv_full = np.zeros((E, VC), f16)
        wvb_np = np.zeros((1, VC), f32)
        for h in range(4):
            ch = slice(g * GC + h * D, g * GC + (h + 1) * D)
            wv_full[:, h * (D + 1) : h * (D + 1) + D] = Wv[ch, :].T
            wvb_np[0, h * (D + 1) : h * (D + 1) + D] = bv[ch]
            wvb_np[0, h * (D + 1) + D] = 1.0  # ones col -> softmax denominator
        wv_t = pmajor(wv_full, VC)

        wo_t = pmajor(Wo[:, cols].T.astype(f16), E)

        padb_np = np.where(kpm[b], -1.0e30, 0.0).astype(f32).reshape(NSB, P).T
        biases = np.stack(
            [bk[cols][:P], bk[cols][P:], bq[cols][:P], bq[cols][P:]], axis=1
        ).astype(f32)
        padb_np = np.ascontiguousarray(np.concatenate([padb_np, biases], axis=1))

        in_maps.append(
            {
                "xq_t": xprep(query[b]),
                "xk_t": xprep(key[b]),
                "xv_t": xprep(value[b]),
                "wq_t": wq_t,
                "wk_t": wk_t,
                "wv_t": wv_t,
                "wvb": wvb_np,
                "wo_t": wo_t,
                "padb": padb_np,
            }
        )
    return in_maps


def kernel(**inputs) -> np.ndarray:
    nc = _get_nc()
    in_maps = _make_in_maps(**inputs)
    res = run_bass_kernel_spmd(nc, in_maps, core_ids=list(range(8)))
    bo = np.asarray(inputs["bo"], np.float32)
    B = inputs["query"].shape[0]
    out = np.zeros((B, T, E), np.float32)
    for c in range(8):
        b = c // 4
        out[b] += res.results[c]["out_t"].T.astype(np.float32)
    out += bo[None, None, :]
    return out

